# revision 1
# baseline (speedup 1.0000x reference)
"""Trainium2 Bass kernel for nn_PositionalEmbedding (embedding-lookup form).

Math: out[b, 2j]   = mean_k sin(params[k] * dc[b,k] * inv_freq[j])
      out[b, 2j+1] = mean_k cos(params[k] * dc[b,k] * inv_freq[j])

dc[b,k] are integers in [0, 60), so sin/cos over all (k, value) pairs form a
360-row lookup table T[k*60+v, 0:512] (sin/cos interleaved, pre-scaled 1/6)
that is built on-device from `params`.  The batch reduction then becomes, per
128-row tile, out_tile = onehotT.T @ T accumulated over 3 K-chunks of 120
dictionary rows, where onehotT[p, b] = (dc[b, k(p)] == v(p)) / 6 is built with
one small replication matmul + one fused DVE is_equal*scale per chunk.

Data parallel over 8 NeuronCores: each core handles 16384 rows.
"""

import numpy as np
import ml_dtypes

B = 131072
D = 512
NCOMP = 6
HYPER = 2100.0
NCORES = 8
BL = B // NCORES          # 16384 rows per core
P = 128                   # partitions / rows per output tile
NV = 60                   # dictionary values per component
ND = NCOMP * NV           # 360 dictionary rows
CK = 120                  # dictionary rows per K-chunk (2 components)
NCHUNK = ND // CK         # 3
GROUP = 4                 # output tiles per one-hot group (512 batch cols)

PI = float(np.pi)
TWO_PI = 2.0 * PI
# Mod-free range reduction (the DVE tensor-scalar ALU forbids MOD):
#   u = phase/(2*pi) + shift,  d = u - int_cast(u),  sin(2*pi*d - pi)
# equals sin(phase) for shift=0.5 and cos(phase) for shift=0.75, for ANY
# integer-rounding mode of the cast (trunc/floor/rne differ by a whole
# number, i.e. by 2*pi in the argument).
SHIFT_SIN = 0.5
SHIFT_COS = 0.75

_CACHE: dict = {}


def _host_constants():
    j = np.arange(0, D, 2, dtype=np.float32)
    inv_freq = np.float32(HYPER) ** (-(np.float32(2.0) * (j + np.float32(1.0))) / np.float32(D))
    # folded 1/(2*pi): the table build works on u = phase/(2*pi)
    scaled = (inv_freq.astype(np.float64) / (2.0 * np.pi)).astype(np.float32)
    invf2 = np.empty((D,), np.float32)
    invf2[0::2] = scaled
    invf2[1::2] = scaled
    invf2b = np.ascontiguousarray(np.broadcast_to(invf2, (CK, D)))

    # replication matrix: R[k, d] = 1 if k == d // NV
    repl = np.zeros((NCOMP, ND), np.float32)
    for k in range(NCOMP):
        repl[k, k * NV:(k + 1) * NV] = 1.0

    vvals = (np.arange(CK, dtype=np.float32) % NV).reshape(CK, 1)
    return invf2b, repl, vvals


def _build_nc(bl):
    import concourse.bacc as bacc
    import concourse.mybir as mybir
    from concourse import tile

    f32 = mybir.dt.float32
    f16 = mybir.dt.bfloat16
    Alu = mybir.AluOpType
    Act = mybir.ActivationFunctionType

    nc = bacc.Bacc(trn_type="TRN2")
    dct = nc.dram_tensor("dct", [NCOMP, bl], f16, kind="ExternalInput").ap()
    pvd = nc.dram_tensor("pvd", [CK, NCHUNK], f32, kind="ExternalInput").ap()
    r16 = nc.dram_tensor("r16", [NCOMP, ND], f16, kind="ExternalInput").ap()
    vvd = nc.dram_tensor("vvd", [CK, 1], f32, kind="ExternalInput").ap()
    ivd = nc.dram_tensor("ivd", [CK, D], f32, kind="ExternalInput").ap()
    out = nc.dram_tensor("out", [bl, D], f32, kind="ExternalOutput").ap()

    ntiles = bl // P
    ngroups = ntiles // GROUP

    with tile.TileContext(nc) as tc:
        with (
            tc.tile_pool(name="const", bufs=1) as cpool,
            tc.tile_pool(name="tbl", bufs=3) as wpool,
            tc.tile_pool(name="oh", bufs=9) as ohpool,
            tc.tile_pool(name="osb", bufs=6) as opool,
            tc.tile_pool(name="crep", bufs=3, space="PSUM") as ppool,
            tc.tile_pool(name="ops", bufs=4, space="PSUM") as qpool,
        ):
            # ---- constants into SBUF
            dct_sb = cpool.tile([NCOMP, bl], f16, tag="dct")
            nc.sync.dma_start(out=dct_sb[:, :], in_=dct)
            pv_sb = cpool.tile([CK, NCHUNK], f32, tag="pv")
            nc.sync.dma_start(out=pv_sb[:, :], in_=pvd)
            r16_sb = cpool.tile([NCOMP, ND], f16, tag="r16")
            nc.sync.dma_start(out=r16_sb[:, :], in_=r16)
            vv_sb = cpool.tile([CK, 1], f32, tag="vv")
            nc.sync.dma_start(out=vv_sb[:, :], in_=vvd)
            if_sb = cpool.tile([CK, D], f32, tag="if2")
            nc.sync.dma_start(out=if_sb[:, :], in_=ivd)
            mpi_sb = cpool.tile([CK, 1], f32, tag="mpi")
            nc.vector.memset(mpi_sb[:, :], -PI)

            # ---- main loop, software-pipelined EMISSION order.
            # Per group g:  main-matmuls(g) -> one-hot(g+1) -> copies+DMAs(g)
            # The is_equal of group g+1 waits on crep(g+1), which sits after
            # all of group g's matmuls in PE program order — so by the time
            # the PSUM->SBUF copies of group g run, DVE's view of the PE
            # clock already covers their matmuls and each copy needs only
            # its single ob-slot WAR (out-DMA) wait.  Every instruction
            # stays within walrus's one-sync-wait-per-instruction limit.
            def emit_onehot(g):
                ohs = []
                for c in range(NCHUNK):
                    crep = ppool.tile([CK, GROUP * P], f32, tag="crep")
                    nc.tensor.matmul(
                        crep[:, :], r16_sb[:, c * CK:(c + 1) * CK],
                        dct_sb[:, g * GROUP * P:(g + 1) * GROUP * P],
                        start=True, stop=True,
                    )
                    oh = ohpool.tile([CK, GROUP * P], f16, tag="oh")
                    nc.vector.tensor_scalar(
                        out=oh[:, :], in0=crep[:, :],
                        scalar1=vv_sb[:, :], scalar2=None,
                        op0=Alu.is_equal,
                    )
                    ohs.append(oh)
                return ohs

            ohs = emit_onehot(0)
            # ---- build sin/cos lookup table, 3 chunks of [120, 512] fp16
            shift_sb = cpool.tile([CK, D], f32, tag="shift")
            nc.vector.memset(shift_sb[:, 0::2], SHIFT_SIN)
            nc.vector.memset(shift_sb[:, 1::2], SHIFT_COS)
            tbl = []
            for c in range(NCHUNK):
                ph = wpool.tile([CK, D], f32, tag="ph")
                nc.vector.tensor_scalar_mul(ph[:, :], if_sb[:, :], pv_sb[:, c:c + 1])
                u = wpool.tile([CK, D], f32, tag="u")
                nc.vector.tensor_add(out=u[:, :], in0=ph[:, :], in1=shift_sb[:, :])
                ni = wpool.tile([CK, D], mybir.dt.int32, tag="ni")
                nc.vector.tensor_copy(out=ni[:, :], in_=u[:, :])
                nf = wpool.tile([CK, D], f32, tag="nf")
                nc.vector.tensor_copy(out=nf[:, :], in_=ni[:, :])
                d = wpool.tile([CK, D], f32, tag="d")
                nc.vector.tensor_sub(out=d[:, :], in0=u[:, :], in1=nf[:, :])
                # d in (-1,1) whatever rounding the cast used; wrap into
                # [0,1) so the Sin argument 2*pi*d - pi stays in [-pi, pi)
                mk = wpool.tile([CK, D], f32, tag="mk")
                nc.vector.tensor_scalar(
                    out=mk[:, :], in0=d[:, :], scalar1=0.0, scalar2=None,
                    op0=Alu.is_lt,
                )
                dw = wpool.tile([CK, D], f32, tag="dw")
                nc.vector.tensor_add(out=dw[:, :], in0=d[:, :], in1=mk[:, :])
                tt = cpool.tile([CK, D], f16, tag=f"tbl{c}")
                nc.scalar.activation(
                    tt[:, :], dw[:, :], Act.Sin, bias=mpi_sb[:, :], scale=TWO_PI
                )
                tbl.append(tt)

            for g in range(ngroups):
                pss = []
                for t in range(GROUP):
                    ps = qpool.tile([P, D], f32, tag="ops")
                    for c in range(NCHUNK):
                        nc.tensor.matmul(
                            ps[:, :], ohs[c][:, t * P:(t + 1) * P], tbl[c][:, :],
                            start=(c == 0), stop=(c == NCHUNK - 1),
                        )
                    pss.append(ps)
                if g + 1 < ngroups:
                    ohs = emit_onehot(g + 1)
                else:
                    # epilogue: advance DVE's PE clock past the last matmul
                    scrf = cpool.tile([P, 1], f32, tag="scrf")
                    nc.vector.tensor_copy(
                        out=scrf[0:1, :], in_=pss[GROUP - 1][0:1, 0:1]
                    )
                for t in range(GROUP):
                    ob = opool.tile([P, D], f32, tag="ob")
                    # 1/6 scale folded here so the one-hot stays an exact
                    # 1.0 in bf16 (halves the bf16 quantization error).
                    # t=0 on DVE (its PE wait is covered by is_eq(g+1) just
                    # before it in DVE program order), t=1..3 on ACT — keeps
                    # every copy engine under PE's ~3.2us/group so PE never
                    # micro-idles (HAM would throttle it to half rate).
                    if t == 0:
                        nc.vector.tensor_scalar_mul(ob[:, :], pss[t][:, :], 1.0 / NCOMP)
                    else:
                        nc.scalar.mul(ob[:, :], pss[t][:, :], 1.0 / NCOMP)
                    r0 = (g * GROUP + t) * P
                    nc.sync.dma_start(out=out[r0:r0 + P, :], in_=ob[:, :])

    # Bacc legalization: splits multi-sync-waits into EventSemaphores
    # (walrus allows at most one wait per instruction), allocates registers.
    nc.compile()
    return nc


def _get_nc(bl=BL):
    key = ("nc", bl)
    if key not in _CACHE:
        _CACHE[key] = _build_nc(bl)
    return _CACHE[key]


def _in_maps(date_components, params):
    dc = np.asarray(date_components).astype(np.int32, copy=False)
    prm = np.asarray(params).astype(np.float32, copy=False).reshape(NCOMP)
    invf2b, repl, vvals = _host_constants()
    r16 = repl.astype(ml_dtypes.bfloat16)
    # pv[p, c] = params[2c + p//60] * (p % 60), exactly the fp32 product the
    # reference forms (marshalling of the 6 params into the 360 dict rows)
    p_idx = np.arange(CK)
    pv = np.empty((CK, NCHUNK), np.float32)
    for c in range(NCHUNK):
        pv[:, c] = prm[2 * c + p_idx // NV] * (p_idx % NV).astype(np.float32)
    maps = []
    for i in range(NCORES):
        shard = dc[i * BL:(i + 1) * BL]
        dct = np.ascontiguousarray(shard.T).astype(ml_dtypes.bfloat16)
        maps.append({
            "dct": dct,
            "pvd": pv,
            "r16": r16,
            "vvd": vvals,
            "ivd": invf2b,
        })
    return maps


def kernel(date_components, params, _trace=False):
    from concourse.bass_utils import run_bass_kernel_spmd

    nc = _get_nc()
    maps = _in_maps(date_components, params)
    res = run_bass_kernel_spmd(
        nc, maps, core_ids=list(range(NCORES)),
        trace=_trace, trace_cores=[0] if _trace else None,
    )
    kernel.last_results = res
    return np.concatenate([r["out"] for r in res.results], axis=0)

